# revision 17
# baseline (speedup 1.0000x reference)
"""Trainium2 Bass kernel for nn_K_attention_MH (sparse_attention).

Per token t (4096 total): X_t in R^{16x64} (heads x head_dim),
  D[i,j] = ||X_i - X_j||^2 ; K = exp(-sigma*D) ; Y = X + K @ X.

Strategy (pure data parallel over 8 cores, 512 tokens/core):
  - Partition layout: 8 tokens/group -> 128 partitions = (t8, h16), free = d64.
  - Exponent E = G - r_i/2 - r_j/2 - S^2*offdiag built in PSUM by exactly TWO
    PE matmuls per group:
      1) Gram:  tts^T @ tts                        (K=64)
      2) aug:   augL^T @ augR                      (K=10)
    where aug row pairs are (r^T, -1/2), (-1/2, r^T), and 8 mask row pairs
    (u_a, v_a) with u_a = +S block indicator, v_a = -S off-block, so the
    contraction itself injects the -r_i/2 - r_j/2 - S^2*offdiag terms.
  - r^T (per-token-head squared norms, laid out along the free axis) comes
    from a GPSIMD partition_all_reduce of sqT = tts*tts — no PE or heavy DVE
    reduction needed.
  - W = exp(2*sigma*E) via one batched ACT per oct (scale = 2*r_sigma).
  - Y PSUM = I @ X + sum_g W_g^T @ X_g  ==  (K+I) @ X  ==  x + K@x, so the
    residual add is folded into PE accumulation and the output DMAs straight
    from PSUM. No DVE work on the output path.
"""

import sys

for p in ("/opt/trn_rl_repo",):
    if p not in sys.path:
        sys.path.insert(0, p)

import numpy as np

import concourse.bass as bass
import concourse.bass_isa as bass_isa
import concourse.mybir as mybir
import concourse.tile as tile
from concourse import bacc
from concourse.bass_utils import run_bass_kernel_spmd
from concourse.masks import make_identity

N_CORES = 8
B, T, C = 2, 2048, 1024
H, HD = 16, 64
TOK = B * T                 # 4096 tokens total
TOK_PER_CORE = TOK // N_CORES  # 512
GROUPS = TOK_PER_CORE // 8     # 64 groups of 8 tokens
OCTS = GROUPS // 8             # 8 "octs" of 8 groups (64 tokens each)

F16 = mybir.dt.float16
F32 = mybir.dt.float32

MASK_S = 2048.0  # S^2 = 4.19e6 >> any |G - r| term; exp(-2*sigma*S^2) == 0


def build_kernel_body(ctx, nc, tc, x, sig, y):
    # DRAM views: token index = o*64 + g*8 + t ; channel = h*64 + d
    x_r = x.rearrange("(o g t) (h d) -> o (t h) g d", o=OCTS, g=8, t=8, h=H, d=HD)
    y_r = y.rearrange("(o g t) (h d) -> o (t h) g d", o=OCTS, g=8, t=8, h=H, d=HD)

    consts = ctx.enter_context(tc.tile_pool(name="consts", bufs=1))
    sb = ctx.enter_context(tc.tile_pool(name="sb", bufs=4))
    ps_t = ctx.enter_context(tc.tile_pool(name="ps_t", bufs=1, space="PSUM"))
    ps_e = ctx.enter_context(tc.tile_pool(name="ps_e", bufs=2, space="PSUM"))
    ps_y = ctx.enter_context(tc.tile_pool(name="ps_y", bufs=1, space="PSUM"))
    ps_r = ctx.enter_context(tc.tile_pool(name="ps_r", bufs=1, space="PSUM"))

    # ---- one-time constants (generated on-chip) ----
    ident = consts.tile([128, 128], F16)
    make_identity(nc, ident)

    # broadcast 2*r_sigma to [128,1] fp32 for the ACT scale operand
    scale2s = consts.tile([128, 1], F32)
    nc.gpsimd.dma_start(out=scale2s, in_=sig[:].to_broadcast((128, 1)))
    nc.vector.tensor_scalar_mul(out=scale2s, in0=scale2s, scalar1=2.0)

    # oct 0's input DMA + cast issued BEFORE the aug-const block so the DVE
    # queue reaches them immediately (the const memsets then overlap with
    # oct 0's transposes/Gram instead of gating them)
    pre_tbig = sb.tile([128, 8, HD], F32, tag="tbig")
    nc.sync.dma_start(out=pre_tbig, in_=x_r[0])
    pre_t16 = sb.tile([128, 8, HD], F16, tag="t16")
    nc.vector.tensor_copy(out=pre_t16, in_=pre_tbig)

    # Augmented-contraction operand tiles, [72, (s2, p4, th128)] fp16.
    # SBUF partition ranges must start 32-aligned, so the three live sections
    # sit at partitions 0, 32, and 64; everything else is zero (and
    # contributes nothing to the PE accumulation):
    #   partition 0:     L = -r^T/2     R = 1
    #   partition 32:    L = 1          R = -r^T/2
    #   partitions 64-71: L = u_a (+S on block a), R = v_a (-S off-block)
    # Mask/const rows are built once; -r/2 rows are rewritten per oct.
    augL = consts.tile([72, 2, 4, 128], F16, name="augL")
    augR = consts.tile([72, 2, 4, 128], F16, name="augR")

    # mask generators built in cheap [8,128] scratches, then broadcast
    masku = consts.tile([8, 128], F16)
    nc.gpsimd.memset(masku, MASK_S)
    nc.gpsimd.affine_select(
        out=masku, in_=masku, compare_op=mybir.AluOpType.is_ge, fill=0.0,
        base=0, pattern=[[1, 128]], channel_multiplier=-16,
    )
    nc.gpsimd.affine_select(
        out=masku, in_=masku, compare_op=mybir.AluOpType.is_ge, fill=0.0,
        base=15, pattern=[[-1, 128]], channel_multiplier=16,
    )
    maskv = consts.tile([8, 128], F16)
    nc.vector.tensor_scalar(
        out=maskv, in0=masku, scalar1=-MASK_S, scalar2=None,
        op0=mybir.AluOpType.add,
    )

    def bc8(t):
        return bass.AP(
            tensor=t.tensor, offset=t.offset,
            ap=[t.ap[0], [0, 2], [0, 4], t.ap[1]],
        )

    nc.vector.memset(augL, 0.0)
    nc.vector.memset(augR, 0.0)
    nc.vector.memset(augL[32:33], 1.0)
    nc.vector.memset(augR[0:1], 1.0)
    nc.vector.tensor_copy(out=augL[64:72], in_=bc8(masku))
    nc.vector.tensor_copy(out=augR[64:72], in_=bc8(maskv))

    # stationary [-1/2] column for the r-matmul: -r^T/2 = neghalf^T @ sqT
    neghalf = consts.tile([128, 1], F16)
    nc.vector.memset(neghalf, -0.5)

    # tiny warmup cast: absorbs the one-time first-CAST setup cost (~3.3us)
    # during the constant phase instead of on oct 0's critical path
    warm16 = consts.tile([128, 1], F16)
    nc.vector.tensor_copy(out=warm16, in_=scale2s)

    # ---- main loop: one oct = 64 tokens = 8 groups ----
    for o in range(OCTS):
        if o == 0:
            tbig, t16 = pre_tbig, pre_t16
        else:
            tbig = sb.tile([128, 8, HD], F32, tag="tbig")
            nc.sync.dma_start(out=tbig, in_=x_r[o])
            t16 = sb.tile([128, 8, HD], F16, tag="t16")
            nc.vector.tensor_copy(out=t16, in_=tbig)

        # transposes: pair p covers groups (2p, 2p+1)
        pst = ps_t.tile([128, 4, 128], F16, tag="pst")
        for p in range(4):
            nc.tensor.transpose(pst[:, p, :], t16[:, 2 * p:2 * p + 2, :], ident)

        tts = sb.tile([128, 4, 128], F16, tag="tts")
        nc.vector.tensor_copy(
            out=tts.bitcast(mybir.dt.uint32), in_=pst.bitcast(mybir.dt.uint32))

        # -r^T/2 rows: square tts on the Scalar engine, column-sum via a
        # cheap M=1 PE matmul with a [-1/2]-ones stationary, then copy the
        # PSUM row into the aug operand tiles (int32 bitcast halves the DVE
        # element count).
        sqT = sb.tile([128, 4, 128], F16, tag="sqT")
        nc.scalar.activation(
            out=sqT, in_=tts, func=mybir.ActivationFunctionType.Square,
            bias=0.0, scale=1.0)
        rps = ps_r.tile([1, 2, 4, 128], F32, tag="rps")
        for s in range(2):
            ksl = slice(64 * s, 64 * (s + 1))
            nc.tensor.matmul(
                rps[:, s], neghalf[ksl], sqT[ksl],
                start=True, stop=True, skip_group_check=True,
            )
        nc.scalar.copy(out=augL[0:1], in_=rps)
        nc.vector.tensor_copy(
            out=augR[32:33].bitcast(mybir.dt.uint32),
            in_=augL[0:1].bitcast(mybir.dt.uint32))

        # E[:, s, p, :] for group 2p+s: Gram (start) + aug (stop) per group.
        # NOTE: start=True clears has_written for the WHOLE PSUM bank, so emit
        # exactly one start=True per bank (the first MM).
        e = ps_e.tile([128, 2, 4, 128], F32, tag="e")
        for s in range(2):
            ksl = slice(64 * s, 64 * (s + 1))
            for p in range(4):
                nc.tensor.matmul(
                    e[:, s, p, :], tts[ksl, p, :], tts[ksl, p, :],
                    start=(p == 0), stop=False, skip_group_check=True,
                )
            for p in range(4):
                nc.tensor.matmul(
                    e[:, s, p, :], augL[:, s, p, :], augR[:, s, p, :],
                    start=False, stop=(p == 3), skip_group_check=True,
                )

        w = sb.tile([128, 2, 4, 128], F16, tag="w")
        nc.scalar.activation(
            out=w, in_=e, func=mybir.ActivationFunctionType.Exp,
            bias=0.0, scale=scale2s[:, 0:1],
        )

        yb = ps_y.tile([128, 8, HD], F32, tag="yb")
        for p in range(4):
            for s in range(2):
                g = 2 * p + s
                nc.tensor.matmul(
                    yb[:, g, :], w[:, s, p, :], t16[:, g, :],
                    start=(g == 0), stop=(g == 7), skip_group_check=True,
                )

        out = sb.tile([128, 8, HD], F32, tag="out")
        nc.vector.tensor_tensor(
            out=out, in0=yb, in1=tbig, op=mybir.AluOpType.add)
        nc.gpsimd.dma_start(out=y_r[o], in_=out)


_NC_CACHE = None


def build_nc():
    global _NC_CACHE
    if _NC_CACHE is not None:
        return _NC_CACHE
    nc = bacc.Bacc("TRN2", target_bir_lowering=False, num_devices=N_CORES)
    x = nc.dram_tensor("x", [TOK_PER_CORE, C], F32, kind="ExternalInput")
    sig = nc.dram_tensor("r_sigma", [1], F32, kind="ExternalInput")
    y = nc.dram_tensor("y", [TOK_PER_CORE, C], F32, kind="ExternalOutput")
    from contextlib import ExitStack
    with tile.TileContext(nc) as tc, ExitStack() as ctx:
        build_kernel_body(ctx, nc, tc, x, sig, y)
    nc.compile()
    _NC_CACHE = nc
    return nc


def kernel(x: np.ndarray, r_sigma: np.ndarray) -> np.ndarray:
    assert x.shape == (B, T, C) and x.dtype == np.float32
    nc = build_nc()
    xf = np.ascontiguousarray(x.reshape(TOK, C))
    sig = np.ascontiguousarray(r_sigma.astype(np.float32))
    in_maps = [
        {"x": xf[k * TOK_PER_CORE:(k + 1) * TOK_PER_CORE], "r_sigma": sig}
        for k in range(N_CORES)
    ]
    res = run_bass_kernel_spmd(nc, in_maps, core_ids=list(range(N_CORES)))
    out = np.concatenate([r["y"] for r in res.results], axis=0)
    return out.reshape(B, T, C)


if __name__ == "__main__":
    x = np.random.default_rng(0).standard_normal((B, T, C)).astype(np.float32)
    r_sigma = np.array([0.01], dtype=np.float32)
    y = kernel(x, r_sigma)
    print("ok", y.shape, y.dtype)
